# revision 10
# baseline (speedup 1.0000x reference)
"""Trainium2 Bass kernel for CosineSimilarityCodebook (vector-quantization) eval forward.

Computes, for x:[4,4096,256] f32 and embeddings:[8192,256] f32:
    flatten = l2norm(x.reshape(-1, 256))          (row scale > 0 -> argmax invariant, skipped)
    emb_n   = l2norm(embeddings)
    dist    = flatten @ emb_n.T
    indices = argmax(dist, axis=-1)               (first occurrence on ties)
    quantized = embeddings[indices]               (UN-normalized codebook gather)
returns (quantized [4,4096,256] f32, indices [4,4096] int32).

Distribution: data-parallel over the flattened token dim N=16384 -> 8 cores x 2048
tokens; the [8192,256] codebook is replicated per core.

Per-core pipeline (all on one NeuronCore, SPMD over 8):
  phase 0: load codebook tiles [128,256], per-row 1/||e|| via ACT Square+accum ->
           Sqrt -> DVE reciprocal, scale rows (DVE), PE-transpose into
           embT[2][128,8192] (d-major), batched ACT copies out of PSUM.
  main, per 128-token tile:
    - PE transposes x tile -> xT [d,tok]
    - 32 matmuls (fp32r) accumulate scores into PSUM quarters [128,2048]
    - DVE tensor_tensor_scan (running max) PSUM -> cum SBUF  (the only full
      DVE pass over score data)
    - gmax = max of quarter-end running maxes (tiny)
    - per quarter: count elements with cummax < gmax (ACT Sign+accum / GPSIMD
      is_lt+accum); sum of counts == first-occurrence argmax index (exact,
      tie-safe: cummax_k < gmax  <=>  k < argmax)
    - index fp32->uint32 (GPSIMD), indirect-DMA row gather from DRAM codebook,
      DMA out.
"""

import numpy as np

TOK_PER_CORE = 2048
N_CORES = 8
D = 256
K = 8192
CHUNK = 512  # codes per matmul (PSUM bank)
QUART = 2048  # codes per PSUM tile / scan
P = 128

_CACHE = {}


def _build(score_dtype="float32r", sign_split=7168, n_tok=TOK_PER_CORE, k=K):
    """Build + compile the per-core Bass module. Returns nc."""
    from contextlib import ExitStack

    import concourse.bass as bass
    import concourse.mybir as mybir
    import concourse.tile as tile
    from concourse import bacc
    from concourse.alu_op_type import AluOpType
    from concourse.masks import make_identity

    f32 = mybir.dt.float32
    sdt = getattr(mybir.dt, score_dtype)
    sign_split = max(1024, min(sign_split, k - 1024))
    n_tiles = n_tok // P
    n_quart = k // QUART
    n_ctiles = k // P  # codebook tiles of 128 rows

    nc = bacc.Bacc(
        "TRN2", target_bir_lowering=False, debug=False, num_devices=N_CORES
    )

    x = nc.dram_tensor("x", [n_tok, D], f32, kind="ExternalInput").ap()
    emb = nc.dram_tensor("emb", [k, D], f32, kind="ExternalInput").ap()
    q_out = nc.dram_tensor("q_out", [n_tok, D], f32, kind="ExternalOutput").ap()
    idx_out = nc.dram_tensor(
        "idx_out", [P, n_tiles], mybir.dt.int32, kind="ExternalOutput"
    ).ap()

    with tile.TileContext(nc) as tc, ExitStack() as ctx:
        const_pool = ctx.enter_context(tc.tile_pool(name="const", bufs=1))
        embT_pool = ctx.enter_context(tc.tile_pool(name="embT", bufs=1))
        embin_pool = ctx.enter_context(tc.tile_pool(name="embin", bufs=6))
        embn_pool = ctx.enter_context(tc.tile_pool(name="embn", bufs=4))
        scratch_pool = ctx.enter_context(tc.tile_pool(name="scratch", bufs=2))
        small_pool = ctx.enter_context(tc.tile_pool(name="small", bufs=8))
        psum_pool = ctx.enter_context(tc.tile_pool(name="ps", bufs=2, space="PSUM"))
        xin_pool = ctx.enter_context(tc.tile_pool(name="xin", bufs=3))
        xT_pool = ctx.enter_context(tc.tile_pool(name="xT", bufs=2))
        cum_pool = ctx.enter_context(tc.tile_pool(name="cum", bufs=2))
        trash_pool = ctx.enter_context(tc.tile_pool(name="trash", bufs=1))
        qsb_pool = ctx.enter_context(tc.tile_pool(name="qsb", bufs=2))
        idx_pool = ctx.enter_context(tc.tile_pool(name="idx", bufs=1))

        ident = const_pool.tile([P, P], f32, name="ident")
        make_identity(nc, ident[:])
        negb = const_pool.tile([P, 1], f32, name="negb")
        nc.gpsimd.memset(negb[:], -3.0e38)

        embT = [
            embT_pool.tile([P, k], sdt, name=f"embT{h}", tag=f"embT{h}")
            for h in range(2)
        ]
        idxu = idx_pool.tile([P, n_tiles], mybir.dt.uint32, name="idxu")
        ss_all = const_pool.tile([P, n_ctiles], f32, name="ss_all")
        nrm_all = const_pool.tile([P, n_ctiles], f32, name="nrm_all")
        inv_all = const_pool.tile([P, n_ctiles], f32, name="inv_all")
        # sign trash buffers (single-buffered; ACT/DVE are in-order anyway)
        trashA = trash_pool.tile([P, sign_split], mybir.dt.int8, name="trashA")
        trashB = trash_pool.tile([P, k - sign_split], mybir.dt.int8, name="trashB")

        # ---- phase 0: normalize + transpose codebook into embT ----
        # groups of 4 code-tiles share two PSUM tiles (4 transposes each)
        for g in range(n_ctiles // 4):
            psA = psum_pool.tile([P, QUART], f32, tag="ps", name="psA")
            psB = psum_pool.tile([P, QUART], f32, tag="ps", name="psB")
            ets = []
            for i in range(4):
                ct = 4 * g + i
                et = embin_pool.tile([P, D], f32, name="et")
                ets.append(et)
                nc.sync.dma_start(et[:], emb[ct * P : (ct + 1) * P, :])
                sq = scratch_pool.tile([P, D], f32, name="sq")
                nc.scalar.activation(
                    sq[:],
                    et[:],
                    mybir.ActivationFunctionType.Square,
                    accum_out=ss_all[:, ct : ct + 1],
                )
            gs = slice(4 * g, 4 * g + 4)
            nc.scalar.activation(
                nrm_all[:, gs], ss_all[:, gs], mybir.ActivationFunctionType.Sqrt
            )
            nc.vector.reciprocal(inv_all[:, gs], nrm_all[:, gs])
            for i in range(4):
                ct = 4 * g + i
                en = embn_pool.tile([P, D], f32, name="en")
                nc.vector.tensor_scalar(
                    out=en[:],
                    in0=ets[i][:],
                    scalar1=inv_all[:, ct : ct + 1],
                    scalar2=None,
                    op0=AluOpType.mult,
                )
                nc.tensor.transpose(
                    psA[:, i * CHUNK : i * CHUNK + P], en[:, 0:P], ident[:]
                )
                nc.tensor.transpose(
                    psB[:, i * CHUNK : i * CHUNK + P], en[:, P:D], ident[:]
                )
            # batched PSUM -> embT copies ([128, 4, 128] strided view)
            for h, ps in ((0, psA), (1, psB)):
                src = ps[:].rearrange("p (a b) -> p a b", b=CHUNK)[:, :, 0:P]
                dst = embT[h][:, g * CHUNK : (g + 1) * CHUNK].rearrange(
                    "p (a b) -> p a b", b=P
                )
                nc.scalar.copy(dst, src)

        # ---- main loop over 128-token tiles ----
        for t in range(n_tiles):
            xt_in = xin_pool.tile([P, D], f32, name="xt_in")
            nc.sync.dma_start(xt_in[:], x[t * P : (t + 1) * P, :])
            psX = psum_pool.tile([P, QUART], f32, tag="ps", name="psX")
            nc.tensor.transpose(psX[:, 0:P], xt_in[:, 0:P], ident[:])
            nc.tensor.transpose(psX[:, CHUNK : CHUNK + P], xt_in[:, P:D], ident[:])
            xT = xT_pool.tile([P, 2 * P], sdt, name="xT")
            nc.scalar.copy(
                xT[:].rearrange("p (a b) -> p a b", b=P),
                psX[:, 0 : 2 * CHUNK].rearrange("p (a b) -> p a b", b=CHUNK)[
                    :, :, 0:P
                ],
            )

            cum = cum_pool.tile([P, k], f32, name="cum")
            knt = small_pool.tile([P, 2], f32, name="knt")

            for q in range(n_quart):
                ps = psum_pool.tile([P, QUART], f32, tag="ps", name="psq")
                for j in range(QUART // CHUNK):
                    c = q * (QUART // CHUNK) + j
                    for h in range(2):
                        nc.tensor.matmul(
                            ps[:, j * CHUNK : (j + 1) * CHUNK],
                            lhsT=xT[:, h * P : (h + 1) * P],
                            rhs=embT[h][:, c * CHUNK : (c + 1) * CHUNK],
                            start=(h == 0),
                            stop=(h == 1),
                        )
                # chain quarters: initial = previous quarter's running max
                init = -3.0e38 if q == 0 else cum[:, q * QUART - 1 : q * QUART]
                nc.vector.tensor_tensor_scan(
                    out=cum[:, q * QUART : (q + 1) * QUART],
                    data0=ps[:],
                    data1=negb[:].to_broadcast([P, QUART]),
                    initial=init,
                    op0=AluOpType.max,
                    op1=AluOpType.max,
                )

            # global max = last element of the chained running max
            gmax = cum[:, k - 1 : k]

            # count of (cummax < gmax) == first-occurrence argmax index.
            # ACT takes [0, sign_split) via Sign(gmax - cummax) + accum;
            # DVE takes the tail via is_lt + accum (2x mode, all-SBUF).
            nc.scalar.activation(
                trashA[:],
                cum[:, 0:sign_split],
                mybir.ActivationFunctionType.Sign,
                bias=gmax,
                scale=-1.0,
                accum_out=knt[:, 0:1],
            )
            nc.vector.tensor_scalar(
                out=trashB[:],
                in0=cum[:, sign_split:k],
                scalar1=gmax,
                scalar2=None,
                op0=AluOpType.is_lt,
                op1=AluOpType.add,
                accum_out=knt[:, 1:2],
            )

            kf = small_pool.tile([P, 1], f32, name="kf")
            nc.vector.tensor_reduce(
                kf[:], knt[:], axis=mybir.AxisListType.X, op=AluOpType.add
            )
            nc.vector.tensor_copy(out=idxu[:, t : t + 1], in_=kf[:])

            qsb = qsb_pool.tile([P, D], f32, name="qsb")
            nc.gpsimd.indirect_dma_start(
                out=qsb[:],
                out_offset=None,
                in_=emb[:],
                in_offset=bass.IndirectOffsetOnAxis(ap=idxu[:, t : t + 1], axis=0),
            )
            nc.sync.dma_start(q_out[t * P : (t + 1) * P, :], qsb[:])

        nc.sync.dma_start(idx_out[:], idxu[:].bitcast(mybir.dt.int32))

    nc.compile()
    return nc


def _get_nc(**kw):
    key = tuple(sorted(kw.items()))
    if key not in _CACHE:
        _CACHE[key] = _build(**kw)
    return _CACHE[key]


LAST_RESULTS = None


def kernel(x: np.ndarray, embeddings: np.ndarray):
    import os

    from concourse.bass_utils import run_bass_kernel_spmd

    global LAST_RESULTS
    shape = x.shape
    d = shape[-1]
    assert d == D and embeddings.shape == (K, D)
    xf = np.ascontiguousarray(x.reshape(-1, d), dtype=np.float32)
    embc = np.ascontiguousarray(embeddings, dtype=np.float32)
    n = xf.shape[0]
    per = n // N_CORES
    assert per == TOK_PER_CORE

    nc = _get_nc()
    in_maps = [
        {"x": xf[c * per : (c + 1) * per], "emb": embc} for c in range(N_CORES)
    ]
    trace = bool(os.environ.get("BASS_KERNEL_TRACE"))
    res = run_bass_kernel_spmd(
        nc, in_maps, core_ids=list(range(N_CORES)), trace=trace
    )
    LAST_RESULTS = res

    quant = np.concatenate(
        [res.results[c]["q_out"] for c in range(N_CORES)], axis=0
    ).reshape(shape)
    # idx_out[p, t] holds the index for token t*128+p of the core's slab
    idx = np.concatenate(
        [res.results[c]["idx_out"].T.reshape(-1) for c in range(N_CORES)], axis=0
    ).astype(np.int32)
    return quant, idx.reshape(shape[:-1])


# revision 18
# speedup vs baseline: 1.0155x; 1.0155x over previous
"""Trainium2 Bass kernel for CosineSimilarityCodebook (vector-quantization) eval forward.

Computes, for x:[4,4096,256] f32 and embeddings:[8192,256] f32:
    flatten = l2norm(x.reshape(-1, 256))          (row scale > 0 -> argmax invariant, skipped)
    emb_n   = l2norm(embeddings)
    dist    = flatten @ emb_n.T
    indices = argmax(dist, axis=-1)               (first occurrence on ties)
    quantized = embeddings[indices]               (UN-normalized codebook gather)
returns (quantized [4,4096,256] f32, indices [4,4096] int32).

Distribution: data-parallel over the flattened token dim N=16384 -> 8 cores x 2048
tokens; the [8192,256] codebook is replicated per core.

Per-core pipeline (all on one NeuronCore, SPMD over 8):
  phase 0: load codebook tiles [128,256], per-row 1/||e|| via ACT Square+accum ->
           Sqrt -> DVE reciprocal, scale rows (DVE), PE-transpose into
           embT[2][128,8192] (d-major), batched ACT copies out of PSUM.
  main, per 128-token tile:
    - PE transposes x tile -> xT [d,tok]
    - 32 matmuls (fp32r) accumulate scores into PSUM quarters [128,2048]
    - DVE tensor_tensor_scan (running max) PSUM -> cum SBUF  (the only full
      DVE pass over score data)
    - gmax = max of quarter-end running maxes (tiny)
    - per quarter: count elements with cummax < gmax (ACT Sign+accum / GPSIMD
      is_lt+accum); sum of counts == first-occurrence argmax index (exact,
      tie-safe: cummax_k < gmax  <=>  k < argmax)
    - index fp32->uint32 (GPSIMD), indirect-DMA row gather from DRAM codebook,
      DMA out.
"""

import numpy as np

TOK_PER_CORE = 2048
N_CORES = 8
D = 256
K = 8192
CHUNK = 512  # codes per matmul (PSUM bank)
QUART = 2048  # codes per PSUM tile / scan
P = 128

_CACHE = {}


def _build(
    score_dtype="float32r",
    sign_split=None,
    scan_bypass=True,
    warmup_mms=20,
    n_tok=TOK_PER_CORE,
    k=K,
):
    """Build + compile the per-core Bass module. Returns nc."""
    from contextlib import ExitStack

    import concourse.bass as bass
    import concourse.mybir as mybir
    import concourse.tile as tile
    from concourse import bacc
    from concourse.alu_op_type import AluOpType
    from concourse.masks import make_identity

    f32 = mybir.dt.float32
    sdt = getattr(mybir.dt, score_dtype)
    if sign_split is None:
        sign_split = k  # whole count on ACT
    sign_split = max(1024, min(sign_split, k))
    n_tiles = n_tok // P
    n_quart = k // QUART
    n_ctiles = k // P  # codebook tiles of 128 rows

    nc = bacc.Bacc(
        "TRN2", target_bir_lowering=False, debug=False, num_devices=N_CORES
    )

    x = nc.dram_tensor("x", [n_tok, D], f32, kind="ExternalInput").ap()
    emb = nc.dram_tensor("emb", [k, D], f32, kind="ExternalInput").ap()
    q_out = nc.dram_tensor("q_out", [n_tok, D], f32, kind="ExternalOutput").ap()
    idx_out = nc.dram_tensor(
        "idx_out", [P, n_tiles], mybir.dt.int32, kind="ExternalOutput"
    ).ap()

    with tile.TileContext(nc) as tc, ExitStack() as ctx:
        const_pool = ctx.enter_context(tc.tile_pool(name="const", bufs=1))
        embT_pool = ctx.enter_context(tc.tile_pool(name="embT", bufs=1))
        embin_pool = ctx.enter_context(tc.tile_pool(name="embin", bufs=6))
        embn_pool = ctx.enter_context(tc.tile_pool(name="embn", bufs=4))
        scratch_pool = ctx.enter_context(tc.tile_pool(name="scratch", bufs=2))
        small_pool = ctx.enter_context(tc.tile_pool(name="small", bufs=8))
        psum_pool = ctx.enter_context(tc.tile_pool(name="ps", bufs=2, space="PSUM"))
        xin_pool = ctx.enter_context(tc.tile_pool(name="xin", bufs=3))
        xT_pool = ctx.enter_context(tc.tile_pool(name="xT", bufs=1))
        cum_pool = ctx.enter_context(tc.tile_pool(name="cum", bufs=2))
        trash_pool = ctx.enter_context(tc.tile_pool(name="trash", bufs=1))
        qsb_pool = ctx.enter_context(tc.tile_pool(name="qsb", bufs=2))
        idx_pool = ctx.enter_context(tc.tile_pool(name="idx", bufs=1))

        ident = const_pool.tile([P, P], f32, name="ident")
        make_identity(nc, ident[:])
        negb = const_pool.tile([P, 1], f32, name="negb")
        nc.gpsimd.memset(negb[:], -3.0e38)

        embT = [
            embT_pool.tile([P, k], sdt, name=f"embT{h}", tag=f"embT{h}")
            for h in range(2)
        ]
        idxu = idx_pool.tile([P, n_tiles], mybir.dt.uint32, name="idxu")
        ss_all = const_pool.tile([P, n_ctiles], f32, name="ss_all")
        nrm_all = const_pool.tile([P, n_ctiles], f32, name="nrm_all")
        inv_all = const_pool.tile([P, n_ctiles], f32, name="inv_all")
        # sign trash buffers (single-buffered; ACT/DVE are in-order anyway)
        trashA = trash_pool.tile([P, sign_split], mybir.dt.int8, name="trashA")
        trashB = (
            trash_pool.tile([P, k - sign_split], mybir.dt.int8, name="trashB")
            if sign_split < k
            else None
        )
        wdummy = const_pool.tile([P, P], f32, name="wdummy")
        nc.gpsimd.memset(wdummy[:], 1.0)

        # ---- phase 0: normalize + transpose codebook into embT ----
        # groups of 4 code-tiles share two PSUM tiles (4 transposes each)
        for g in range(n_ctiles // 4):
            psA = psum_pool.tile([P, QUART], f32, tag="ps", name="psA")
            psB = psum_pool.tile([P, QUART], f32, tag="ps", name="psB")
            ets = []
            for i in range(4):
                ct = 4 * g + i
                et = embin_pool.tile([P, D], f32, name="et")
                ets.append(et)
                nc.sync.dma_start(et[:], emb[ct * P : (ct + 1) * P, :])
                sq = scratch_pool.tile([P, D], f32, name="sq")
                nc.scalar.activation(
                    sq[:],
                    et[:],
                    mybir.ActivationFunctionType.Square,
                    accum_out=ss_all[:, ct : ct + 1],
                )
            gs = slice(4 * g, 4 * g + 4)
            nc.scalar.activation(
                nrm_all[:, gs], ss_all[:, gs], mybir.ActivationFunctionType.Sqrt
            )
            nc.vector.reciprocal(inv_all[:, gs], nrm_all[:, gs])
            for i in range(4):
                ct = 4 * g + i
                en = embn_pool.tile([P, D], f32, name="en")
                nc.vector.tensor_scalar(
                    out=en[:],
                    in0=ets[i][:],
                    scalar1=inv_all[:, ct : ct + 1],
                    scalar2=None,
                    op0=AluOpType.mult,
                )
                nc.tensor.transpose(
                    psA[:, i * CHUNK : i * CHUNK + P], en[:, 0:P], ident[:]
                )
                nc.tensor.transpose(
                    psB[:, i * CHUNK : i * CHUNK + P], en[:, P:D], ident[:]
                )
            # batched PSUM -> embT copies ([128, 4, 128] strided view)
            for h, ps in ((0, psA), (1, psB)):
                src = ps[:].rearrange("p (a b) -> p a b", b=CHUNK)[:, :, 0:P]
                dst = embT[h][:, g * CHUNK : (g + 1) * CHUNK].rearrange(
                    "p (a b) -> p a b", b=P
                )
                nc.scalar.copy(dst, src)

        # ---- prefetch + transpose ALL x tiles (keeps main loop PE-dense) ----
        xTs = []
        for t in range(n_tiles):
            xt_in = xin_pool.tile([P, D], f32, name="xt_in")
            nc.sync.dma_start(xt_in[:], x[t * P : (t + 1) * P, :])
            psX = psum_pool.tile([P, QUART], f32, tag="ps", name="psX")
            nc.tensor.transpose(psX[:, 0:P], xt_in[:, 0:P], ident[:])
            nc.tensor.transpose(psX[:, CHUNK : CHUNK + P], xt_in[:, P:D], ident[:])
            xT = xT_pool.tile([P, 2 * P], sdt, name="xT", tag=f"xT{t}")
            nc.scalar.copy(
                xT[:].rearrange("p (a b) -> p a b", b=P),
                psX[:, 0 : 2 * CHUNK].rearrange("p (a b) -> p a b", b=CHUNK)[
                    :, :, 0:P
                ],
            )
            xTs.append(xT)

        # ---- PE warmup: dense fp32 matmuls to disengage the HAM throttle ----
        if warmup_mms:
            ps_w = psum_pool.tile([P, QUART], f32, tag="ps", name="ps_w")
            for w in range(warmup_mms):
                nc.tensor.matmul(
                    ps_w[:, 0:P],
                    lhsT=ident[:],
                    rhs=wdummy[:],
                    start=True,
                    stop=True,
                )
            warm_trash = small_pool.tile([P, 1], f32, name="warm_trash")
            nc.vector.tensor_reduce(
                warm_trash[:],
                ps_w[:, 0:P],
                axis=mybir.AxisListType.X,
                op=AluOpType.max,
            )

        # ---- main loop over 128-token tiles ----
        for t in range(n_tiles):
            xT = xTs[t]
            cum = cum_pool.tile([P, k], f32, name="cum")
            knt = small_pool.tile([P, 2], f32, name="knt")

            for q in range(n_quart):
                ps = psum_pool.tile([P, QUART], f32, tag="ps", name="psq")
                for j in range(QUART // CHUNK):
                    c = q * (QUART // CHUNK) + j
                    for h in range(2):
                        nc.tensor.matmul(
                            ps[:, j * CHUNK : (j + 1) * CHUNK],
                            lhsT=xT[:, h * P : (h + 1) * P],
                            rhs=embT[h][:, c * CHUNK : (c + 1) * CHUNK],
                            start=(h == 0),
                            stop=(h == 1),
                        )
                # chain quarters: initial = previous quarter's running max
                init = -3.0e38 if q == 0 else cum[:, q * QUART - 1 : q * QUART]
                nc.vector.tensor_tensor_scan(
                    out=cum[:, q * QUART : (q + 1) * QUART],
                    data0=ps[:],
                    data1=negb[:].to_broadcast([P, QUART]),
                    initial=init,
                    op0=AluOpType.max,
                    op1=AluOpType.bypass if scan_bypass else AluOpType.max,
                )

            # global max = last element of the chained running max
            gmax = cum[:, k - 1 : k]

            # count of (cummax < gmax) == first-occurrence argmax index.
            # ACT takes [0, sign_split) via Sign(gmax - cummax) + accum;
            # DVE takes the remaining tail via is_lt + accum.
            kf = small_pool.tile([P, 1], f32, name="kf")
            if sign_split >= k:
                nc.scalar.activation(
                    trashA[:],
                    cum[:],
                    mybir.ActivationFunctionType.Sign,
                    bias=gmax,
                    scale=-1.0,
                    accum_out=kf[:],
                )
            else:
                nc.scalar.activation(
                    trashA[:],
                    cum[:, 0:sign_split],
                    mybir.ActivationFunctionType.Sign,
                    bias=gmax,
                    scale=-1.0,
                    accum_out=knt[:, 0:1],
                )
                nc.vector.tensor_scalar(
                    out=trashB[:],
                    in0=cum[:, sign_split:k],
                    scalar1=gmax,
                    scalar2=None,
                    op0=AluOpType.is_lt,
                    op1=AluOpType.add,
                    accum_out=knt[:, 1:2],
                )
                nc.vector.tensor_reduce(
                    kf[:], knt[:], axis=mybir.AxisListType.X, op=AluOpType.add
                )
            nc.vector.tensor_copy(out=idxu[:, t : t + 1], in_=kf[:])

            qsb = qsb_pool.tile([P, D], f32, name="qsb")
            nc.gpsimd.indirect_dma_start(
                out=qsb[:],
                out_offset=None,
                in_=emb[:],
                in_offset=bass.IndirectOffsetOnAxis(ap=idxu[:, t : t + 1], axis=0),
            )
            nc.sync.dma_start(q_out[t * P : (t + 1) * P, :], qsb[:])

        nc.sync.dma_start(idx_out[:], idxu[:].bitcast(mybir.dt.int32))

    nc.compile()
    return nc


def _get_nc(**kw):
    key = tuple(sorted(kw.items()))
    if key not in _CACHE:
        _CACHE[key] = _build(**kw)
    return _CACHE[key]


LAST_RESULTS = None


def kernel(x: np.ndarray, embeddings: np.ndarray):
    import os

    from concourse.bass_utils import run_bass_kernel_spmd

    global LAST_RESULTS
    shape = x.shape
    d = shape[-1]
    assert d == D and embeddings.shape == (K, D)
    xf = np.ascontiguousarray(x.reshape(-1, d), dtype=np.float32)
    embc = np.ascontiguousarray(embeddings, dtype=np.float32)
    n = xf.shape[0]
    per = n // N_CORES
    assert per == TOK_PER_CORE

    nc = _get_nc()
    in_maps = [
        {"x": xf[c * per : (c + 1) * per], "emb": embc} for c in range(N_CORES)
    ]
    trace = bool(os.environ.get("BASS_KERNEL_TRACE"))
    res = run_bass_kernel_spmd(
        nc, in_maps, core_ids=list(range(N_CORES)), trace=trace
    )
    LAST_RESULTS = res

    quant = np.concatenate(
        [res.results[c]["q_out"] for c in range(N_CORES)], axis=0
    ).reshape(shape)
    # idx_out[p, t] holds the index for token t*128+p of the core's slab
    idx = np.concatenate(
        [res.results[c]["idx_out"].T.reshape(-1) for c in range(N_CORES)], axis=0
    ).astype(np.int32)
    return quant, idx.reshape(shape[:-1])
